# revision 34
# baseline (speedup 1.0000x reference)
"""Trainium2 kernel for nn_Aleat5_1ChamferL2NormalWeightedALLSVDLoss.

Reference semantics: a block (a ~10% slab in x of the margin-trimmed pred
bounding box) selects pred points (ind_p) and target points (ind_t).  For
every pred point, d = min squared L2 distance to a *selected* target; the
loss only reads d at *selected* pred points (keep = ind_p & (d < kth)).
So the only heavy compute that affects the output is the exact NN search
of selected preds (~1450/batch) against selected targets (~1450/batch).

Device plan (8 NeuronCores, SPMD): core i = (batch b = i//2, half of b's
selected preds, sorted by y).  d[c, m] = p2[c] - 2 p.t + t2[m] is a K=5
fp32 matmul (lhsT rows = [px, py, pz, p2, 1], rhs rows =
[-2tx, -2ty, -2tz, 1, t2]) into PSUM, then reduce-min over targets on the
Vector engine.  Matmuls of consecutive pred chunks are packed into
different 32-row PE tile groups (tile_position) so up to 4 run
concurrently.  Targets are pre-pruned per chunk to a y-window
[chunk_ymin - B, chunk_ymax + B] where B is a host-computed upper bound
on every pred's NN distance — provably containing the true NN, so the
device min is exact.  Host does the cheap O(N) selection, the kth-value
cutoff and the final scalar reduction, mirroring the reference in fp32.
"""

import numpy as np

BLOCK_SIZE = (0.1, 1.0, 1.0)
BLOCK_IDX = (0, 0, 0)
MARGIN = 0.05
RATIO = 0.5
GAMMA = 0.0
BIG = 1e30
N_CORES = 8
NGRP = 4  # concurrent 32-row PE tile groups
F32 = np.float32

_NC_CACHE = {}
_RUN_KWARGS = {}  # test.py sets {"trace": True, ...} to profile
LAST_RESULTS = None


def _selection(pred, target):
    """ind_p, ind_t exactly as the reference computes them (fp32)."""
    mins = pred.min(axis=1)
    maxs = pred.max(axis=1)
    width = maxs - mins
    lo = mins + width * F32(MARGIN)
    hi = maxs - width * F32(MARGIN)
    bs = np.asarray(BLOCK_SIZE, F32)
    idx = np.asarray(BLOCK_IDX, F32)
    bmin = lo + (hi - lo) * idx * bs
    bmax = bmin + (hi - lo) * bs
    ind_p = np.all((bmin[:, None, :] < pred) & (pred < bmax[:, None, :]), axis=-1)
    ind_t = np.all((bmin[:, None, :] < target) & (target < bmax[:, None, :]), axis=-1)
    return ind_p, ind_t


def _nn_upper_bound(sp, st):
    """Upper bound on max_p min_t ||p - t||, in float64, via a target
    subsample plus y-neighbourhood candidates.  Any valid upper bound
    keeps the device result exact; a tighter one just shrinks windows."""
    sp64 = sp.astype(np.float64)
    st64 = st.astype(np.float64)
    sub = st64[::4]
    d2 = ((sp64[:, None, :] - sub[None, :, :]) ** 2).sum(-1).min(1)
    order = np.argsort(st64[:, 1], kind="stable")
    sty = st64[order]
    pos = np.searchsorted(sty[:, 1], sp64[:, 1])
    K = 16
    for k in range(-K, K):
        idx = np.clip(pos + k, 0, len(sty) - 1)
        cand = sty[idx]
        d2 = np.minimum(d2, ((sp64 - cand) ** 2).sum(-1))
    return float(np.sqrt(d2.max())) * 1.05 + 1e-4


def _final_loss(d_sel, mk_sel, counts, alpha, B):
    """Per-batch kth-value cutoff + weighted mean + focal term (fp32)."""
    loss_b = np.zeros(B, F32)
    for b in range(B):
        dsel = d_sel[b]
        count = counts[b]
        if count == 0:
            loss_b[b] = F32(0.0)
            continue
        kidx = int(np.int32(F32(count) * F32(RATIO)))
        m = np.sort(dsel)[kidx]
        keep = dsel < m
        mk = mk_sel[b]
        sq = np.where(keep, dsel * dsel, F32(0.0)) * mk
        denom = F32(np.sum(np.where(keep, mk, F32(0.0)))) + F32(1e-12)
        loss_b[b] = F32(np.sum(sq)) / denom
    loss = F32(np.mean(loss_b))
    alpha = alpha.astype(F32)
    ea = np.exp(-alpha) * loss
    fw = ea ** F32(GAMMA)
    fw = fw / (F32(np.sum(fw)) + F32(1e-12))
    loss = F32(np.sum(fw * ea)) + alpha[0]
    return np.asarray(loss, dtype=F32)


def _numpy_fallback(pred, target, mask, alpha, ind_p, ind_t):
    """Faithful numpy port of the full reference (degenerate cases only)."""
    B, N, _ = pred.shape
    t2 = np.sum(target * target, -1) + np.where(ind_t, F32(0.0), F32(BIG))
    p2 = np.sum(pred * pred, -1)
    d = np.empty((B, N), F32)
    CH = 500
    for b in range(B):
        for s in range(0, N, CH):
            p = pred[b, s : s + CH]
            dd = (
                p2[b, s : s + CH, None]
                + t2[b][None, :]
                - F32(2.0) * (p @ target[b].T)
            )
            d[b, s : s + CH] = dd.min(-1)
    d = np.maximum(d, F32(0.0))
    d_m = np.where(ind_p, d, F32(BIG))
    mk = mask[..., 0]
    loss_b = np.zeros(B, F32)
    for b in range(B):
        count = int(ind_p[b].sum())
        kidx = int(np.int32(F32(count) * F32(RATIO)))
        m = np.sort(d_m[b])[kidx]
        keep = ind_p[b] & (d[b] < m)
        sq = np.where(keep, d[b] * d[b], F32(0.0)) * mk[b]
        denom = F32(np.sum(np.where(keep, mk[b], F32(0.0)))) + F32(1e-12)
        loss_b[b] = F32(np.sum(sq)) / denom
    loss = F32(np.mean(loss_b))
    alpha = alpha.astype(F32)
    ea = np.exp(-alpha) * loss
    fw = ea ** F32(GAMMA)
    fw = fw / (F32(np.sum(fw)) + F32(1e-12))
    loss = F32(np.sum(fw * ea)) + alpha[0]
    return np.asarray(loss, dtype=F32)


def _build_nc(PP, NCHUNK, T, npack, W):
    """NCHUNK pred chunks of 128; per chunk T target tiles of W (<=512);
    matmul k = c*T + t runs in 32-row group k % NGRP, rhs tile at column
    block k // NGRP.  PSUM: one [128, W*T] tile per chunk, min-reduced to
    one column of the output."""
    import concourse.tile as tile
    from concourse import bacc, mybir

    nc = bacc.Bacc(
        "TRN2",
        target_bir_lowering=False,
        debug=False,
        num_devices=N_CORES,
    )
    f32 = mybir.dt.float32
    NPART = 32 * (NGRP - 1) + 5
    COLS = PP + W * npack  # per-group row: [lhs | rhs tiles]
    inp = nc.dram_tensor("inp", [5 * NGRP, COLS], f32, kind="ExternalInput").ap()
    out = nc.dram_tensor("out", [128, NCHUNK], f32, kind="ExternalOutput").ap()

    psum_bufs = max(2, min(4, 8 // T))
    half = (NCHUNK + 1) // 2
    with tile.TileContext(nc) as tc:
        with (
            tc.tile_pool(name="io", bufs=1) as io_pool,
            tc.tile_pool(name="ps", bufs=psum_bufs, space="PSUM") as psum_pool,
        ):
            inp_sb = io_pool.tile([NPART, COLS], f32)
            dout = io_pool.tile([128, NCHUNK], f32)
            dma_engines = [nc.sync, nc.gpsimd, nc.scalar]
            for g in range(NGRP):
                eng = dma_engines[g % len(dma_engines)]
                eng.dma_start(
                    out=inp_sb[32 * g : 32 * g + 5, :],
                    in_=inp[5 * g : 5 * g + 5, :],
                )
            for c in range(NCHUNK):
                ps = psum_pool.tile([128, W * T], f32)
                for t in range(T):
                    k = c * T + t
                    g, j = k % NGRP, k // NGRP
                    nc.tensor.matmul(
                        out=ps[:, W * t : W * (t + 1)],
                        lhsT=inp_sb[32 * g : 32 * g + 5, 128 * c : 128 * (c + 1)],
                        rhs=inp_sb[
                            32 * g : 32 * g + 5,
                            PP + W * j : PP + W * (j + 1),
                        ],
                        start=True,
                        stop=True,
                        tile_position=(32 * g, 0),
                    )
                nc.vector.tensor_reduce(
                    out=dout[:, c : c + 1],
                    in_=ps[:, :],
                    axis=mybir.AxisListType.X,
                    op=mybir.AluOpType.min,
                )
                # overlap half the result writeback with remaining chunks
                if c == half - 1:
                    nc.sync.dma_start(out=out[:, :half], in_=dout[:, :half])
            nc.sync.dma_start(out=out[:, half:], in_=dout[:, half:])
    nc.compile()
    return nc


def _build_nc_raw(PP, NCHUNK, T, npack, W):
    """Raw-bass variant of _build_nc: manual semaphores, no Tile
    preamble/teardown barriers.  Semaphores are cleared by gpsimd as the
    globally-last operation (gated on the output DMAs) so the NEFF can be
    re-executed."""
    import concourse.bass as bass
    from concourse import mybir

    nc = bass.Bass("TRN2", target_bir_lowering=False, debug=False)
    f32 = mybir.dt.float32
    NPART = 32 * (NGRP - 1) + 5
    COLS = PP + W * npack
    inp = nc.dram_tensor("inp", [5 * NGRP, COLS], f32, kind="ExternalInput").ap()
    out = nc.dram_tensor("out", [128, NCHUNK], f32, kind="ExternalOutput").ap()

    NBUF = max(2, min(4, (8 - T) // T))
    half = (NCHUNK + 1) // 2
    with (
        nc.sbuf_tensor([NPART, COLS], f32) as inp_sb,
        nc.sbuf_tensor([128, NCHUNK], f32) as dout,
        nc.psum_tensor([128, NBUF, W * T], f32) as ps,
        nc.psum_tensor([128, W], f32) as ps_warm,
        nc.sbuf_tensor([37, W], f32) as warm_sb,
        nc.semaphore("sd0") as sd0,
        nc.semaphore("sd1") as sd1,
        nc.semaphore("sd2") as sd2,
        nc.semaphore("sd3") as sd3,
        nc.semaphore("smm") as smm,
        nc.semaphore("sred") as sred,
        nc.semaphore("swarm") as swarm,
        nc.semaphore("sdo") as sdo,
        nc.Block() as block,
    ):
        dsems = [sd0, sd1, sd2, sd3]
        sem_range = range(sd0.num, sdo.num + 1)

        split_out = half >= 2 and NCHUNK - half >= 2

        @block.sync
        def _(sync):
            sync.dma_start(
                out=inp_sb[0:5, :], in_=inp[0:5, :]
            ).then_inc(sd0, 16)
            g = 3 % NGRP
            sync.dma_start(
                out=inp_sb[32 * g : 32 * g + 5, :], in_=inp[5 * g : 5 * g + 5, :]
            ).then_inc(dsems[g], 16)
            if split_out:
                sync.wait_ge(sred, half)
                sync.dma_start(out=out[:, :half], in_=dout[:, :half]).then_inc(
                    sdo, 16
                )
                sync.wait_ge(sred, NCHUNK)
                sync.dma_start(out=out[:, half:], in_=dout[:, half:]).then_inc(
                    sdo, 16
                )
            else:
                sync.wait_ge(sred, NCHUNK)
                sync.dma_start(out=out[:, :], in_=dout[:, :]).then_inc(sdo, 16)
                sync.sem_inc(sdo, 16)

        @block.gpsimd
        def _(gpsimd):
            if NGRP > 1:
                gpsimd.dma_start(
                    out=inp_sb[32:37, :], in_=inp[5:10, :]
                ).then_inc(sd1, 16)
            # globally-last: everything (incl. output DMAs) feeds sdo
            gpsimd.wait_ge(sdo, 32)

        @block.scalar
        def _(scalar):
            if NGRP > 2:
                scalar.dma_start(
                    out=inp_sb[64:69, :], in_=inp[10:15, :]
                ).then_inc(sd2, 16)

        @block.tensor
        def _(tensor):
            # dummy matmuls on scratch SBUF while the input DMAs run: keeps
            # the PE busy >3.4us so HAM un-throttles 1.2->2.4GHz before the
            # real matmuls arrive
            tensor.wait_ge(swarm, 1)
            for _w in range(6):
                nc.tensor.matmul(
                    out=ps_warm[:, :],
                    lhsT=warm_sb[32:37, 0:128],
                    rhs=warm_sb[32:37, 0:W],
                    start=True,
                    stop=True,
                    tile_position=(32, 0),
                )
            seen = set()
            for c in range(NCHUNK):
                if c >= NBUF:
                    tensor.wait_ge(sred, c - NBUF + 1)
                for t in range(T):
                    k = c * T + t
                    g, j = k % NGRP, k // NGRP
                    if g not in seen:
                        seen.add(g)
                        tensor.wait_ge(dsems[g], 16)
                    mm = nc.tensor.matmul(
                        out=ps[:, c % NBUF, W * t : W * (t + 1)],
                        lhsT=inp_sb[32 * g : 32 * g + 5, 128 * c : 128 * (c + 1)],
                        rhs=inp_sb[
                            32 * g : 32 * g + 5, PP + W * j : PP + W * (j + 1)
                        ],
                        start=True,
                        stop=True,
                        tile_position=(32 * g, 0),
                    )
                    if t == T - 1:
                        mm.then_inc(smm, 1)

        @block.vector
        def _(vector):
            nc.vector.memset(warm_sb[:, :], 0.0).then_inc(swarm, 1)
            for c in range(NCHUNK):
                vector.wait_ge(smm, c + 1)
                nc.vector.tensor_reduce(
                    out=dout[:, c : c + 1],
                    in_=ps[:, c % NBUF, :],
                    axis=mybir.AxisListType.X,
                    op=mybir.AluOpType.min,
                ).then_inc(sred, 1)

        # teardown: one barrier (cheap — every other engine is already done
        # when gpsimd's sdo wait clears), then reset sems for re-execution
        nc.all_engine_barrier(sem_only=True)
        nc.gpsimd.dma_reset(sem_range)
        nc.gpsimd.sem_clear(sem_range)

    return nc


def kernel(pred, target, mask, alpha):
    pred = np.asarray(pred, F32)
    target = np.asarray(target, F32)
    mask = np.asarray(mask, F32)
    alpha = np.asarray(alpha, F32)
    B = pred.shape[0]

    ind_p, ind_t = _selection(pred, target)
    counts_p = ind_p.sum(1)
    counts_t = ind_t.sum(1)
    if counts_p.min() == 0 or counts_t.min() == 0 or B * 2 != N_CORES:
        return _numpy_fallback(pred, target, mask, alpha, ind_p, ind_t)

    # --- per-core shards: core i = (batch i//2, y-sorted pred half i%2) ---
    shards = []  # per core: (sorted pred pts, orig idx, sorted tgt pts, windows)
    NCHUNK = 0
    Wmax = 1
    for b in range(B):
        sel_idx = np.nonzero(ind_p[b])[0]
        sp = pred[b, sel_idx]
        order = np.argsort(sp[:, 1], kind="stable")
        sp, sel_idx = sp[order], sel_idx[order]
        st = target[b, np.nonzero(ind_t[b])[0]]
        st = st[np.argsort(st[:, 1], kind="stable")]
        Bb = _nn_upper_bound(sp, st)
        ty = st[:, 1].astype(np.float64)
        h = (len(sp) + 1) // 2
        for lo, hi in ((0, h), (h, len(sp))):
            ppts, pidx = sp[lo:hi], sel_idx[lo:hi]
            nch = max(1, (len(ppts) + 127) // 128)
            NCHUNK = max(NCHUNK, nch)
            wins = []
            for c in range(nch):
                ch = ppts[128 * c : 128 * (c + 1), 1].astype(np.float64)
                wlo = int(np.searchsorted(ty, ch.min() - Bb, "left"))
                whi = int(np.searchsorted(ty, ch.max() + Bb, "right"))
                wins.append((wlo, whi))
                Wmax = max(Wmax, whi - wlo)
            shards.append((ppts, pidx, st, wins))

    PP = 128 * NCHUNK
    T = (Wmax + 511) // 512
    W = 512
    npack = (NCHUNK * T + NGRP - 1) // NGRP

    in_maps = []
    for i in range(N_CORES):
        ppts, pidx, st, wins = shards[i]
        p2 = ppts[:, 0] ** 2 + ppts[:, 1] ** 2 + ppts[:, 2] ** 2
        n = len(ppts)
        t2 = st[:, 0] ** 2 + st[:, 1] ** 2 + st[:, 2] ** 2
        inp_i = np.zeros((5 * NGRP, PP + W * npack), F32)
        for g in range(NGRP):
            r = 5 * g
            inp_i[r + 0, :n] = ppts[:, 0]
            inp_i[r + 1, :n] = ppts[:, 1]
            inp_i[r + 2, :n] = ppts[:, 2]
            inp_i[r + 3, :n] = p2
            inp_i[r + 4, :n] = F32(1.0)
            inp_i[r + 4, PP:] = F32(BIG)  # rhs padding never wins min
        for c in range(NCHUNK):
            wlo, whi = wins[c] if c < len(wins) else wins[-1]
            for t in range(T):
                a = wlo + W * t
                bnd = min(a + W, whi)
                if a >= bnd:
                    continue
                w = bnd - a
                k = c * T + t
                r, col = 5 * (k % NGRP), PP + W * (k // NGRP)
                inp_i[r + 0, col : col + w] = F32(-2.0) * st[a:bnd, 0]
                inp_i[r + 1, col : col + w] = F32(-2.0) * st[a:bnd, 1]
                inp_i[r + 2, col : col + w] = F32(-2.0) * st[a:bnd, 2]
                inp_i[r + 3, col : col + w] = F32(1.0)
                inp_i[r + 4, col : col + w] = t2[a:bnd]
        in_maps.append({"inp": inp_i})

    key = (PP, NCHUNK, T, npack, W)
    if key not in _NC_CACHE:
        try:
            _NC_CACHE[key] = _build_nc_raw(PP, NCHUNK, T, npack, W)
        except Exception:
            _NC_CACHE[key] = _build_nc(PP, NCHUNK, T, npack, W)
    nc = _NC_CACHE[key]

    from concourse.bass_utils import run_bass_kernel_spmd

    res = run_bass_kernel_spmd(nc, in_maps, list(range(N_CORES)), **_RUN_KWARGS)
    globals()["LAST_RESULTS"] = res

    # --- gather: out[p, c] = d for the (c*128 + p)-th y-sorted pred ---
    d_sel = []
    mk_sel = []
    for b in range(B):
        parts = []
        midx = []
        for half in range(2):
            i = 2 * b + half
            ppts, pidx, _, _ = shards[i]
            dcore = np.asarray(res.results[i]["out"], F32)  # [128, NCHUNK]
            parts.append(dcore.T.reshape(-1)[: len(ppts)])
            midx.append(pidx)
        d_b = np.maximum(np.concatenate(parts), F32(0.0))
        d_sel.append(d_b)
        mk_sel.append(mask[b, np.concatenate(midx), 0])

    globals()["DEBUG_D"] = (d_sel, [np.concatenate([shards[2 * b][1], shards[2 * b + 1][1]]) for b in range(B)])
    return _final_loss(d_sel, mk_sel, counts_p, alpha, B)


# revision 39
# speedup vs baseline: 1.3230x; 1.3230x over previous
"""Trainium2 kernel for nn_Aleat5_1ChamferL2NormalWeightedALLSVDLoss.

Reference semantics: a block (a ~10% slab in x of the margin-trimmed pred
bounding box) selects pred points (ind_p) and target points (ind_t).  For
every pred point, d = min squared L2 distance to a *selected* target; the
loss only reads d at *selected* pred points (keep = ind_p & (d < kth)).
So the only heavy compute that affects the output is the exact NN search
of selected preds (~1450/batch) against selected targets (~1450/batch).

Device plan (8 NeuronCores, SPMD): core i = (batch b = i//2, half of b's
selected preds, sorted by y).  d[c, m] = p2[c] - 2 p.t + t2[m] is a K=5
fp32 matmul (lhsT rows = [px, py, pz, p2, 1], rhs rows =
[-2tx, -2ty, -2tz, 1, t2]) into PSUM, then reduce-min over targets on the
Vector engine.  Matmuls of consecutive pred chunks are packed into
different 32-row PE tile groups (tile_position) so up to 4 run
concurrently.  Targets are pre-pruned per chunk to a y-window
[chunk_ymin - B, chunk_ymax + B] where B is a host-computed upper bound
on every pred's NN distance — provably containing the true NN, so the
device min is exact.  Host does the cheap O(N) selection, the kth-value
cutoff and the final scalar reduction, mirroring the reference in fp32.
"""

import numpy as np

BLOCK_SIZE = (0.1, 1.0, 1.0)
BLOCK_IDX = (0, 0, 0)
MARGIN = 0.05
RATIO = 0.5
GAMMA = 0.0
BIG = 1e30
N_CORES = 8
NGRP = 4  # concurrent 32-row PE tile groups
F32 = np.float32

_NC_CACHE = {}
_RUN_KWARGS = {}  # test.py sets {"trace": True, ...} to profile
LAST_RESULTS = None


def _selection(pred, target):
    """ind_p, ind_t exactly as the reference computes them (fp32)."""
    mins = pred.min(axis=1)
    maxs = pred.max(axis=1)
    width = maxs - mins
    lo = mins + width * F32(MARGIN)
    hi = maxs - width * F32(MARGIN)
    bs = np.asarray(BLOCK_SIZE, F32)
    idx = np.asarray(BLOCK_IDX, F32)
    bmin = lo + (hi - lo) * idx * bs
    bmax = bmin + (hi - lo) * bs
    ind_p = np.all((bmin[:, None, :] < pred) & (pred < bmax[:, None, :]), axis=-1)
    ind_t = np.all((bmin[:, None, :] < target) & (target < bmax[:, None, :]), axis=-1)
    return ind_p, ind_t


def _nn_upper_bound(sp, st):
    """Upper bound on max_p min_t ||p - t||, in float64, via a target
    subsample plus y-neighbourhood candidates.  Any valid upper bound
    keeps the device result exact; a tighter one just shrinks windows."""
    sp64 = sp.astype(np.float64)
    st64 = st.astype(np.float64)
    sub = st64[::4]
    d2 = ((sp64[:, None, :] - sub[None, :, :]) ** 2).sum(-1).min(1)
    order = np.argsort(st64[:, 1], kind="stable")
    sty = st64[order]
    pos = np.searchsorted(sty[:, 1], sp64[:, 1])
    K = 16
    for k in range(-K, K):
        idx = np.clip(pos + k, 0, len(sty) - 1)
        cand = sty[idx]
        d2 = np.minimum(d2, ((sp64 - cand) ** 2).sum(-1))
    return float(np.sqrt(d2.max())) * 1.05 + 1e-4


def _final_loss(d_sel, mk_sel, counts, alpha, B):
    """Per-batch kth-value cutoff + weighted mean + focal term (fp32)."""
    loss_b = np.zeros(B, F32)
    for b in range(B):
        dsel = d_sel[b]
        count = counts[b]
        if count == 0:
            loss_b[b] = F32(0.0)
            continue
        kidx = int(np.int32(F32(count) * F32(RATIO)))
        m = np.sort(dsel)[kidx]
        keep = dsel < m
        mk = mk_sel[b]
        sq = np.where(keep, dsel * dsel, F32(0.0)) * mk
        denom = F32(np.sum(np.where(keep, mk, F32(0.0)))) + F32(1e-12)
        loss_b[b] = F32(np.sum(sq)) / denom
    loss = F32(np.mean(loss_b))
    alpha = alpha.astype(F32)
    ea = np.exp(-alpha) * loss
    fw = ea ** F32(GAMMA)
    fw = fw / (F32(np.sum(fw)) + F32(1e-12))
    loss = F32(np.sum(fw * ea)) + alpha[0]
    return np.asarray(loss, dtype=F32)


def _numpy_fallback(pred, target, mask, alpha, ind_p, ind_t):
    """Faithful numpy port of the full reference (degenerate cases only)."""
    B, N, _ = pred.shape
    t2 = np.sum(target * target, -1) + np.where(ind_t, F32(0.0), F32(BIG))
    p2 = np.sum(pred * pred, -1)
    d = np.empty((B, N), F32)
    CH = 500
    for b in range(B):
        for s in range(0, N, CH):
            p = pred[b, s : s + CH]
            dd = (
                p2[b, s : s + CH, None]
                + t2[b][None, :]
                - F32(2.0) * (p @ target[b].T)
            )
            d[b, s : s + CH] = dd.min(-1)
    d = np.maximum(d, F32(0.0))
    d_m = np.where(ind_p, d, F32(BIG))
    mk = mask[..., 0]
    loss_b = np.zeros(B, F32)
    for b in range(B):
        count = int(ind_p[b].sum())
        kidx = int(np.int32(F32(count) * F32(RATIO)))
        m = np.sort(d_m[b])[kidx]
        keep = ind_p[b] & (d[b] < m)
        sq = np.where(keep, d[b] * d[b], F32(0.0)) * mk[b]
        denom = F32(np.sum(np.where(keep, mk[b], F32(0.0)))) + F32(1e-12)
        loss_b[b] = F32(np.sum(sq)) / denom
    loss = F32(np.mean(loss_b))
    alpha = alpha.astype(F32)
    ea = np.exp(-alpha) * loss
    fw = ea ** F32(GAMMA)
    fw = fw / (F32(np.sum(fw)) + F32(1e-12))
    loss = F32(np.sum(fw * ea)) + alpha[0]
    return np.asarray(loss, dtype=F32)


def _build_nc(PP, NCHUNK, T, npack, W):
    """NCHUNK pred chunks of 128; per chunk T target tiles of W (<=512);
    matmul k = c*T + t runs in 32-row group k % NGRP, rhs tile at column
    block k // NGRP.  PSUM: one [128, W*T] tile per chunk, min-reduced to
    one column of the output."""
    import concourse.tile as tile
    from concourse import bacc, mybir

    nc = bacc.Bacc(
        "TRN2",
        target_bir_lowering=False,
        debug=False,
        num_devices=N_CORES,
    )
    f32 = mybir.dt.float32
    NPART = 32 * (NGRP - 1) + 5
    COLS = PP + W * npack  # per-group row: [lhs | rhs tiles]
    inp = nc.dram_tensor("inp", [5 * NGRP, COLS], f32, kind="ExternalInput").ap()
    out = nc.dram_tensor("out", [128, NCHUNK], f32, kind="ExternalOutput").ap()

    psum_bufs = max(2, min(4, 8 // T))
    half = (NCHUNK + 1) // 2
    with tile.TileContext(nc) as tc:
        with (
            tc.tile_pool(name="io", bufs=1) as io_pool,
            tc.tile_pool(name="ps", bufs=psum_bufs, space="PSUM") as psum_pool,
        ):
            inp_sb = io_pool.tile([NPART, COLS], f32)
            dout = io_pool.tile([128, NCHUNK], f32)
            dma_engines = [nc.sync, nc.gpsimd, nc.scalar]
            for g in range(NGRP):
                eng = dma_engines[g % len(dma_engines)]
                eng.dma_start(
                    out=inp_sb[32 * g : 32 * g + 5, :],
                    in_=inp[5 * g : 5 * g + 5, :],
                )
            for c in range(NCHUNK):
                ps = psum_pool.tile([128, W * T], f32)
                for t in range(T):
                    k = c * T + t
                    g, j = k % NGRP, k // NGRP
                    nc.tensor.matmul(
                        out=ps[:, W * t : W * (t + 1)],
                        lhsT=inp_sb[32 * g : 32 * g + 5, 128 * c : 128 * (c + 1)],
                        rhs=inp_sb[
                            32 * g : 32 * g + 5,
                            PP + W * j : PP + W * (j + 1),
                        ],
                        start=True,
                        stop=True,
                        tile_position=(32 * g, 0),
                    )
                nc.vector.tensor_reduce(
                    out=dout[:, c : c + 1],
                    in_=ps[:, :],
                    axis=mybir.AxisListType.X,
                    op=mybir.AluOpType.min,
                )
                # overlap half the result writeback with remaining chunks
                if c == half - 1:
                    nc.sync.dma_start(out=out[:, :half], in_=dout[:, :half])
            nc.sync.dma_start(out=out[:, half:], in_=dout[:, half:])
    nc.compile()
    return nc


def _build_nc_raw(PP, NCHUNK, T, npack, W):
    """Raw-bass variant of _build_nc: manual semaphores, no Tile
    preamble/teardown barriers.  Semaphores are cleared by gpsimd as the
    globally-last operation (gated on the output DMAs) so the NEFF can be
    re-executed."""
    import concourse.bass as bass
    from concourse import mybir

    nc = bass.Bass("TRN2", target_bir_lowering=False, debug=False)
    f32 = mybir.dt.float32
    NPART = 32 * (NGRP - 1) + 5
    COLS = PP + W * npack
    inp = nc.dram_tensor("inp", [5 * NGRP, COLS], f32, kind="ExternalInput").ap()
    out = nc.dram_tensor("out", [128, NCHUNK], f32, kind="ExternalOutput").ap()

    NBUF = max(2, min(4, 8 // T))
    half = (NCHUNK + 1) // 2
    with (
        nc.sbuf_tensor([NPART, COLS], f32) as inp_sb,
        nc.sbuf_tensor([128, NCHUNK], f32) as dout,
        nc.psum_tensor([128, NBUF, W * T], f32) as ps,

        nc.semaphore("sd0") as sd0,
        nc.semaphore("sd1") as sd1,
        nc.semaphore("sd2") as sd2,
        nc.semaphore("sd3") as sd3,
        nc.semaphore("smm") as smm,
        nc.semaphore("sred") as sred,
        nc.semaphore("sdo") as sdo,
        nc.Block() as block,
    ):
        dsems = [sd0, sd1, sd2, sd3]
        sem_range = range(sd0.num, sdo.num + 1)

        split_out = half >= 2 and NCHUNK - half >= 2

        @block.sync
        def _(sync):
            sync.dma_start(
                out=inp_sb[0:5, :], in_=inp[0:5, :]
            ).then_inc(sd0, 16)
            g = 3 % NGRP
            sync.dma_start(
                out=inp_sb[32 * g : 32 * g + 5, :], in_=inp[5 * g : 5 * g + 5, :]
            ).then_inc(dsems[g], 16)
            if split_out:
                sync.wait_ge(sred, half)
                sync.dma_start(out=out[:, :half], in_=dout[:, :half]).then_inc(
                    sdo, 16
                )
                sync.wait_ge(sred, NCHUNK)
                sync.dma_start(out=out[:, half:], in_=dout[:, half:]).then_inc(
                    sdo, 16
                )
            else:
                sync.wait_ge(sred, NCHUNK)
                sync.dma_start(out=out[:, :], in_=dout[:, :]).then_inc(sdo, 16)
                sync.sem_inc(sdo, 16)

        @block.gpsimd
        def _(gpsimd):
            if NGRP > 1:
                gpsimd.dma_start(
                    out=inp_sb[32:37, :], in_=inp[5:10, :]
                ).then_inc(sd1, 16)
            # globally-last: everything (incl. output DMAs) feeds sdo
            gpsimd.wait_ge(sdo, 32)

        @block.scalar
        def _(scalar):
            if NGRP > 2:
                scalar.dma_start(
                    out=inp_sb[64:69, :], in_=inp[10:15, :]
                ).then_inc(sd2, 16)

        @block.tensor
        def _(tensor):
            seen = set()
            for c in range(NCHUNK):
                if c >= NBUF:
                    tensor.wait_ge(sred, c - NBUF + 1)
                for t in range(T):
                    k = c * T + t
                    g, j = k % NGRP, k // NGRP
                    if g not in seen:
                        seen.add(g)
                        tensor.wait_ge(dsems[g], 16)
                    mm = nc.tensor.matmul(
                        out=ps[:, c % NBUF, W * t : W * (t + 1)],
                        lhsT=inp_sb[32 * g : 32 * g + 5, 128 * c : 128 * (c + 1)],
                        rhs=inp_sb[
                            32 * g : 32 * g + 5, PP + W * j : PP + W * (j + 1)
                        ],
                        start=True,
                        stop=True,
                        tile_position=(32 * g, 0),
                    )
                    if t == T - 1:
                        mm.then_inc(smm, 1)

        @block.vector
        def _(vector):
            for c in range(NCHUNK):
                vector.wait_ge(smm, c + 1)
                nc.vector.tensor_reduce(
                    out=dout[:, c : c + 1],
                    in_=ps[:, c % NBUF, :],
                    axis=mybir.AxisListType.X,
                    op=mybir.AluOpType.min,
                ).then_inc(sred, 1)

        # teardown: one barrier (cheap — every other engine is already done
        # when gpsimd's sdo wait clears), then reset sems for re-execution
        nc.all_engine_barrier(sem_only=True)
        nc.gpsimd.dma_reset(sem_range)
        nc.gpsimd.sem_clear(sem_range)

    return nc


def kernel(pred, target, mask, alpha):
    pred = np.asarray(pred, F32)
    target = np.asarray(target, F32)
    mask = np.asarray(mask, F32)
    alpha = np.asarray(alpha, F32)
    B = pred.shape[0]

    ind_p, ind_t = _selection(pred, target)
    counts_p = ind_p.sum(1)
    counts_t = ind_t.sum(1)
    if counts_p.min() == 0 or counts_t.min() == 0 or B * 2 != N_CORES:
        return _numpy_fallback(pred, target, mask, alpha, ind_p, ind_t)

    # --- per-core shards: core i = (batch i//2, y-sorted pred half i%2) ---
    shards = []  # per core: (sorted pred pts, orig idx, sorted tgt pts, windows)
    NCHUNK = 0
    Wmax = 1
    for b in range(B):
        sel_idx = np.nonzero(ind_p[b])[0]
        sp = pred[b, sel_idx]
        order = np.argsort(sp[:, 1], kind="stable")
        sp, sel_idx = sp[order], sel_idx[order]
        st = target[b, np.nonzero(ind_t[b])[0]]
        st = st[np.argsort(st[:, 1], kind="stable")]
        Bb = _nn_upper_bound(sp, st)
        ty = st[:, 1].astype(np.float64)
        h = (len(sp) + 1) // 2
        for lo, hi in ((0, h), (h, len(sp))):
            ppts, pidx = sp[lo:hi], sel_idx[lo:hi]
            nch = max(1, (len(ppts) + 127) // 128)
            NCHUNK = max(NCHUNK, nch)
            wins = []
            for c in range(nch):
                ch = ppts[128 * c : 128 * (c + 1), 1].astype(np.float64)
                wlo = int(np.searchsorted(ty, ch.min() - Bb, "left"))
                whi = int(np.searchsorted(ty, ch.max() + Bb, "right"))
                wins.append((wlo, whi))
                Wmax = max(Wmax, whi - wlo)
            shards.append((ppts, pidx, st, wins))

    PP = 128 * NCHUNK
    T = (Wmax + 511) // 512
    W = 512
    npack = (NCHUNK * T + NGRP - 1) // NGRP

    in_maps = []
    for i in range(N_CORES):
        ppts, pidx, st, wins = shards[i]
        p2 = ppts[:, 0] ** 2 + ppts[:, 1] ** 2 + ppts[:, 2] ** 2
        n = len(ppts)
        t2 = st[:, 0] ** 2 + st[:, 1] ** 2 + st[:, 2] ** 2
        inp_i = np.zeros((5 * NGRP, PP + W * npack), F32)
        for g in range(NGRP):
            r = 5 * g
            inp_i[r + 0, :n] = ppts[:, 0]
            inp_i[r + 1, :n] = ppts[:, 1]
            inp_i[r + 2, :n] = ppts[:, 2]
            inp_i[r + 3, :n] = p2
            inp_i[r + 4, :n] = F32(1.0)
            inp_i[r + 4, PP:] = F32(BIG)  # rhs padding never wins min
        for c in range(NCHUNK):
            wlo, whi = wins[c] if c < len(wins) else wins[-1]
            for t in range(T):
                a = wlo + W * t
                bnd = min(a + W, whi)
                if a >= bnd:
                    continue
                w = bnd - a
                k = c * T + t
                r, col = 5 * (k % NGRP), PP + W * (k // NGRP)
                inp_i[r + 0, col : col + w] = F32(-2.0) * st[a:bnd, 0]
                inp_i[r + 1, col : col + w] = F32(-2.0) * st[a:bnd, 1]
                inp_i[r + 2, col : col + w] = F32(-2.0) * st[a:bnd, 2]
                inp_i[r + 3, col : col + w] = F32(1.0)
                inp_i[r + 4, col : col + w] = t2[a:bnd]
        in_maps.append({"inp": inp_i})

    key = (PP, NCHUNK, T, npack, W)
    if key not in _NC_CACHE:
        try:
            _NC_CACHE[key] = _build_nc_raw(PP, NCHUNK, T, npack, W)
        except Exception:
            _NC_CACHE[key] = _build_nc(PP, NCHUNK, T, npack, W)
    nc = _NC_CACHE[key]

    from concourse.bass_utils import run_bass_kernel_spmd

    res = run_bass_kernel_spmd(nc, in_maps, list(range(N_CORES)), **_RUN_KWARGS)
    globals()["LAST_RESULTS"] = res

    # --- gather: out[p, c] = d for the (c*128 + p)-th y-sorted pred ---
    d_sel = []
    mk_sel = []
    for b in range(B):
        parts = []
        midx = []
        for half in range(2):
            i = 2 * b + half
            ppts, pidx, _, _ = shards[i]
            dcore = np.asarray(res.results[i]["out"], F32)  # [128, NCHUNK]
            parts.append(dcore.T.reshape(-1)[: len(ppts)])
            midx.append(pidx)
        d_b = np.maximum(np.concatenate(parts), F32(0.0))
        d_sel.append(d_b)
        mk_sel.append(mask[b, np.concatenate(midx), 0])

    globals()["DEBUG_D"] = (d_sel, [np.concatenate([shards[2 * b][1], shards[2 * b + 1][1]]) for b in range(B)])
    return _final_loss(d_sel, mk_sel, counts_p, alpha, B)
